# revision 1
# baseline (speedup 1.0000x reference)
"""Sequence-parallel cross-attention kernel for 8 TRN2 NeuronCores.

Math (per reference): Q = in1 @ Wq; K = in2 @ Wk; V = in2 @ Wv;
out = softmax(Q @ K^T) @ V  (no 1/sqrt(d) scaling).

Strategy: shard input_1 rows (queries) across 8 cores, 512 rows each.
All compute is kept q-sharded by reassociating:
    S   = (Q @ Wk^T) @ in2^T          (U := Q @ Wk^T, [512, 1024] per core)
    out = ((A @ in2) @ Wv) / Z        (A := exp(S - max), row sums Z folded at the end)
so no core ever computes the full K or V projection (no replicated big GEMMs,
no collectives).

Numerics: the logits S have sigma ~ 1.6e5 with near-one-hot softmax rows, so S
must be fp32-accurate.  All S-path GEMMs run on the PE in float32r (fp32
rounded to 11 mantissa bits, 1 cycle/row -- 4x faster than fp32) using an
error-compensated hi/lo split: x = hi + lo with both halves f32r, and
A@B ~= Ah@Bh + Ah@Bl + Al@Bh (3 matmuls, fp32-level accuracy, PSUM accumulates
in fp32).  Post-softmax GEMMs run single f32r (errors there are not amplified).
"""

import sys
import numpy as np

sys.path.insert(0, "/opt/trn_rl_repo")

N_CORES = 8
NQ = 4096          # query rows (input_1)
NK = 4096          # key rows (input_2)
D = 1024           # model dim (= d_kq = d_v)
QS = NQ // N_CORES  # 512 query rows per core
P = 128            # partitions
DC = D // P        # 8 chunks of the model dim
QT = QS // P       # 4 query tiles of 128 per core
KC = NK // 512     # 8 key chunks of 512
KCC = NK // P      # 32 key chunks of 128

_compiled = None   # cached Bass program
_last_results = None  # BassKernelResults of the most recent run (for test harness)


def _round_f32r(x):
    """Round fp32 to f32r: round-to-nearest-even at 11 explicit mantissa bits."""
    x = np.ascontiguousarray(x, np.float32)
    u = x.view(np.uint32)
    low = u & np.uint32(0xFFF)
    half = np.uint32(0x800)
    base = u & np.uint32(0xFFFFF000)
    round_up = (low > half) | ((low == half) & (((u >> np.uint32(12)) & np.uint32(1)) == 1))
    return (base + np.where(round_up, np.uint32(0x1000), np.uint32(0)).astype(np.uint32)).view(np.float32)


def _split_f32r(x):
    hi = _round_f32r(x)
    lo = _round_f32r((np.asarray(x, np.float32) - hi).astype(np.float32))
    return hi, lo


def _build(chain_io=False):
    from contextlib import ExitStack
    from concourse import bass, mybir, tile, bacc
    from concourse.masks import make_identity

    F32 = mybir.dt.float32
    F32R = mybir.dt.float32r
    EXP = mybir.ActivationFunctionType.Exp
    COPY = mybir.ActivationFunctionType.Copy

    nc = bacc.Bacc("TRN2", target_bir_lowering=False, debug=False,
                   num_devices=N_CORES)

    # DRAM inputs (per core). f32r tensors carry host-pre-rounded fp32 bits.
    i1t_h = nc.dram_tensor("i1t_h", [D, QS], F32R, kind="ExternalInput").ap()
    i1t_l = nc.dram_tensor("i1t_l", [D, QS], F32R, kind="ExternalInput").ap()
    wq_h = nc.dram_tensor("wq_h", [D, D], F32R, kind="ExternalInput").ap()
    wq_l = nc.dram_tensor("wq_l", [D, D], F32R, kind="ExternalInput").ap()
    wkt_h = nc.dram_tensor("wkt_h", [D, D], F32R, kind="ExternalInput").ap()
    wkt_l = nc.dram_tensor("wkt_l", [D, D], F32R, kind="ExternalInput").ap()
    i2t_h = nc.dram_tensor("i2t_h", [D, NK], F32R, kind="ExternalInput").ap()
    i2t_l = nc.dram_tensor("i2t_l", [D, NK], F32R, kind="ExternalInput").ap()
    i2n = nc.dram_tensor("i2n", [NK, D], F32R, kind="ExternalInput").ap()
    wv = nc.dram_tensor("wv", [D, D], F32R, kind="ExternalInput").ap()
    out_ap = nc.dram_tensor("out", [QS, D], F32, kind="ExternalOutput").ap()
    if chain_io:
        # tiny passthrough tensor for benchmark chaining (defeats CSE when N
        # executions are chained inside one jit; see test.py)
        chain_in = nc.dram_tensor("chain_in", [P, 4], F32, kind="ExternalInput").ap()
        chain_out = nc.dram_tensor("chain_out", [P, 4], F32, kind="ExternalOutput").ap()

    with tile.TileContext(nc, pool_alloc_mode="stack") as tc:
        with ExitStack() as ctx:
            const_pool = ctx.enter_context(tc.tile_pool(name="const", bufs=1))
            stat_pool = ctx.enter_context(tc.tile_pool(name="stat", bufs=1))

            ident = const_pool.tile([P, P], F32, name="ident")
            make_identity(nc, ident[:])
            if chain_io:
                cht = const_pool.tile([P, 4], F32, name="cht")
                nc.sync.dma_start(cht[:], chain_in[:])
                nc.sync.dma_start(chain_out[:], cht[:])

            # SBUF pools form a FIFO chain (Q->U->S->AT->T2); open/close them
            # manually in that order (queue allocator handles the ring reuse).
            q_cm = tc.tile_pool(name="qpool", bufs=1); q_pool = q_cm.__enter__()
            qh = [q_pool.tile([P, QS], F32R, tag=f"qh{d}", name=f"qh{d}") for d in range(DC)]
            ql = [q_pool.tile([P, QS], F32R, tag=f"ql{d}", name=f"ql{d}") for d in range(DC)]

            # ---------------- Phase A: Q^T = 3-term f32r (Wq x in1T) ----------
            with tc.tile_pool(name="wk_a", bufs=1) as wa_pool:
                wqh = [wa_pool.tile([P, D], F32R, tag=f"wqh{d}", name=f"wqh{d}") for d in range(DC)]
                wql = [wa_pool.tile([P, D], F32R, tag=f"wql{d}", name=f"wql{d}") for d in range(DC)]
                i1h = [wa_pool.tile([P, QS], F32R, tag=f"i1h{d}", name=f"i1h{d}") for d in range(DC)]
                i1l = [wa_pool.tile([P, QS], F32R, tag=f"i1l{d}", name=f"i1l{d}") for d in range(DC)]
                for d in range(DC):
                    nc.sync.dma_start(wqh[d][:], wq_h[d * P:(d + 1) * P, :])
                    nc.sync.dma_start(wql[d][:], wq_l[d * P:(d + 1) * P, :])
                    nc.sync.dma_start(i1h[d][:], i1t_h[d * P:(d + 1) * P, :])
                    nc.sync.dma_start(i1l[d][:], i1t_l[d * P:(d + 1) * P, :])
                # di-outer with 8 concurrent accumulators: PE starts as soon as
                # the first weight chunk lands instead of waiting for all of Wq.
                with tc.tile_pool(name="ps_a", bufs=1, space="PSUM") as ps_a:
                    psa = [ps_a.tile([P, QS], F32, tag=f"pa{do}", name=f"pa{do}") for do in range(DC)]
                    for di in range(DC):
                        for do in range(DC):
                            wsl_h = wqh[di][:, do * P:(do + 1) * P]
                            wsl_l = wql[di][:, do * P:(do + 1) * P]
                            for t, (w, x) in enumerate(((wsl_h, i1h[di]), (wsl_h, i1l[di]), (wsl_l, i1h[di]))):
                                nc.tensor.matmul(psa[do][:], w, x[:],
                                                 start=(di == 0 and t == 0),
                                                 stop=(di == DC - 1 and t == 2))
                    for do in range(DC):
                        nc.scalar.copy(qh[do][:], psa[do][:])
                        nc.vector.tensor_sub(ql[do][:], psa[do][:], qh[do][:].bitcast(F32))

            # ---------------- Phase B: U^T = 3-term f32r (Wk^T x Q^T) ---------
            u_cm = tc.tile_pool(name="upool", bufs=1, side="right"); u_pool = u_cm.__enter__()
            uh = [u_pool.tile([P, QS], F32R, tag=f"uh{d}", name=f"uh{d}") for d in range(DC)]
            ul = [u_pool.tile([P, QS], F32R, tag=f"ul{d}", name=f"ul{d}") for d in range(DC)]
            with tc.tile_pool(name="wk_b", bufs=1) as wb_pool:
                wkh = [wb_pool.tile([P, D], F32R, tag=f"wkh{d}", name=f"wkh{d}") for d in range(DC)]
                wkl = [wb_pool.tile([P, D], F32R, tag=f"wkl{d}", name=f"wkl{d}") for d in range(DC)]
                for d in range(DC):
                    nc.sync.dma_start(wkh[d][:], wkt_h[d * P:(d + 1) * P, :])
                    nc.sync.dma_start(wkl[d][:], wkt_l[d * P:(d + 1) * P, :])
                # do-outer (contraction) with 8 concurrent accumulators
                with tc.tile_pool(name="ps_b", bufs=1, space="PSUM") as ps_b:
                    psb = [ps_b.tile([P, QS], F32, tag=f"pb{di}", name=f"pb{di}") for di in range(DC)]
                    for do in range(DC):
                        for di in range(DC):
                            wsl_h = wkh[do][:, di * P:(di + 1) * P]
                            wsl_l = wkl[do][:, di * P:(di + 1) * P]
                            for t, (w, x) in enumerate(((wsl_h, qh[do]), (wsl_h, ql[do]), (wsl_l, qh[do]))):
                                nc.tensor.matmul(psb[di][:], w, x[:],
                                                 start=(do == 0 and t == 0),
                                                 stop=(do == DC - 1 and t == 2))
                    for di in range(DC):
                        nc.scalar.copy(uh[di][:], psb[di][:])
                        nc.vector.tensor_sub(ul[di][:], psb[di][:], uh[di][:].bitcast(F32))
            q_cm.__exit__(None, None, None)  # Q dead after phase B

            # ---------------- Phase C: S (+ softmax) --------------------------
            # S[qt] tiles [128, 4096] fp32; k-chunk outer so each in2T slice is
            # loaded exactly once and reused by all 4 q-tiles.
            s_cm = tc.tile_pool(name="spool", bufs=1); s_pool = s_cm.__enter__()
            s_sb = [s_pool.tile([P, NK], F32, tag=f"s{qt}", name=f"s{qt}") for qt in range(QT)]
            negmax = [stat_pool.tile([P, 1], F32, tag=f"nm{qt}", name=f"nm{qt}") for qt in range(QT)]
            colmax = [stat_pool.tile([P, KC], F32, tag=f"cm{qt}", name=f"cm{qt}") for qt in range(QT)]
            recip_z = [stat_pool.tile([P, 1], F32, tag=f"rz{qt}", name=f"rz{qt}") for qt in range(QT)]
            # open the transpose psum pool BEFORE ps_c so phase D's transposes
            # don't wait on ps_c's pool release (disjoint PSUM banks)
            ps_tr_cm = tc.tile_pool(name="pstr", bufs=4, space="PSUM")
            ps_tr = ps_tr_cm.__enter__()
            with tc.tile_pool(name="ps_c", bufs=4, space="PSUM") as ps_c, \
                 tc.tile_pool(name="i2t_stream", bufs=2) as st_pool:
                for kc in range(KC):
                    ch = [st_pool.tile([P, 512], F32R, tag=f"ch{d}", name=f"ch{d}") for d in range(DC)]
                    cl = [st_pool.tile([P, 512], F32R, tag=f"cl{d}", name=f"cl{d}") for d in range(DC)]
                    for d in range(DC):
                        nc.sync.dma_start(ch[d][:], i2t_h[d * P:(d + 1) * P, kc * 512:(kc + 1) * 512])
                        nc.sync.dma_start(cl[d][:], i2t_l[d * P:(d + 1) * P, kc * 512:(kc + 1) * 512])
                    for qt in range(QT):
                        psum = ps_c.tile([P, 512], F32, tag="mm")
                        n3 = 3 * DC
                        i = 0
                        for di in range(DC):
                            usl_h = uh[di][:, qt * P:(qt + 1) * P]
                            usl_l = ul[di][:, qt * P:(qt + 1) * P]
                            for (w, x) in ((usl_h, ch[di]), (usl_h, cl[di]), (usl_l, ch[di])):
                                nc.tensor.matmul(psum[:], w, x[:], start=(i == 0), stop=(i == n3 - 1))
                                i += 1
                        nc.scalar.copy(s_sb[qt][:, kc * 512:(kc + 1) * 512], psum[:])
                        # pipelined row-max: reduce each 512-chunk as it lands so
                        # only a [128,KC] reduce remains at the phase boundary
                        nc.vector.reduce_max(colmax[qt][:, kc:kc + 1], psum[:],
                                             axis=mybir.AxisListType.X)
            u_cm.__exit__(None, None, None)  # U dead after S GEMMs

            # softmax per q-tile: in-place exp(S - rowmax), row sums
            for qt in range(QT):
                nc.vector.reduce_max(negmax[qt][:], colmax[qt][:], axis=mybir.AxisListType.X)
                nc.vector.tensor_scalar_mul(negmax[qt][:], negmax[qt][:], -1.0)
                zsum = stat_pool.tile([P, 1], F32, tag=f"z{qt}", name=f"z{qt}")
                nc.scalar.activation(s_sb[qt][:], s_sb[qt][:], EXP,
                                     bias=negmax[qt][:], accum_out=zsum[:])
                nc.vector.reciprocal(recip_z[qt][:], zsum[:])

            # ---------------- Phase D: A^T via PE transpose -------------------
            # AT layout: [128, KCC*512]; block kcc holds A^T[kcc*128:+128, 0:512]
            at_cm = tc.tile_pool(name="atpool", bufs=1, side="right"); at_pool = at_cm.__enter__()
            at_all = at_pool.tile([P, KCC * 512], F32R, tag="at", name="at_all")
            for kcc in range(KCC):
                for qt in range(QT):
                    pt = ps_tr.tile([P, P], F32, tag="tr")
                    nc.tensor.transpose(pt[:], s_sb[qt][:, kcc * P:(kcc + 1) * P], ident[:])
                    nc.vector.tensor_copy(at_all[:, kcc * 512 + qt * P: kcc * 512 + (qt + 1) * P], pt[:])
            ps_tr_cm.__exit__(None, None, None)
            s_cm.__exit__(None, None, None)  # A (in S tiles) dead after transposes

            # ---------------- Phase E: T2^T = (A @ in2)^T ---------------------
            # lhsT = in2 native block [k 128, din 128], rhs = A^T [k 128, q 512].
            # kcc-outer with 8 simultaneous dn accumulators (all 8 PSUM banks) so
            # each in2 tile is consumed as soon as it lands.
            t2_cm = tc.tile_pool(name="t2pool", bufs=1); t2_pool = t2_cm.__enter__()
            t2 = [t2_pool.tile([P, QS], F32R, tag=f"t2{d}", name=f"t2{d}") for d in range(DC)]
            ps_e_cm = tc.tile_pool(name="ps_e", bufs=1, space="PSUM")
            ps_e = ps_e_cm.__enter__()
            with tc.tile_pool(name="i2n_stream", bufs=4) as i2n_pool:
                psums = [ps_e.tile([P, QS], F32, tag=f"pe{dn}", name=f"pe{dn}") for dn in range(DC)]
                for kcc in range(KCC):
                    t = i2n_pool.tile([P, D], F32R, tag="i2nb", name="i2nb")
                    nc.sync.dma_start(t[:], i2n[kcc * P:(kcc + 1) * P, :])
                    for dn in range(DC):
                        nc.tensor.matmul(psums[dn][:], t[:, dn * P:(dn + 1) * P],
                                         at_all[:, kcc * 512:(kcc + 1) * 512],
                                         start=(kcc == 0), stop=(kcc == KCC - 1))
                for dn in range(DC):
                    nc.scalar.copy(t2[dn][:], psums[dn][:])
            at_cm.__exit__(None, None, None)

            # ---------------- Phase F: out = (T2 @ Wv) / Z --------------------
            # psums reuse ps_e's bank slots (freed as E's accumulators evict)
            with tc.tile_pool(name="wv_pool", bufs=1) as wv_pool, \
                 tc.tile_pool(name="out_pool", bufs=2) as out_pool:
                wvt = [wv_pool.tile([P, D], F32R, tag=f"wv{d}", name=f"wv{d}") for d in range(DC)]
                for d in range(DC):
                    nc.sync.dma_start(wvt[d][:], wv[d * P:(d + 1) * P, :])
                j = 0
                for qt in range(QT):
                    ot = out_pool.tile([P, D], F32, tag="ot", name="ot")
                    for half in range(2):
                        psum = ps_e.tile([P, 512], F32, tag=f"pe{j % DC}", name=f"pf{j}")
                        j += 1
                        for dn in range(DC):
                            nc.tensor.matmul(psum[:], t2[dn][:, qt * P:(qt + 1) * P],
                                             wvt[dn][:, half * 512:(half + 1) * 512],
                                             start=(dn == 0), stop=(dn == DC - 1))
                        nc.scalar.activation(ot[:, half * 512:(half + 1) * 512], psum[:],
                                             COPY, scale=recip_z[qt][:])
                    nc.sync.dma_start(out_ap[qt * P:(qt + 1) * P, :], ot[:])
            t2_cm.__exit__(None, None, None)
            ps_e_cm.__exit__(None, None, None)

    nc.compile()
    return nc


def _prep_inputs(input_1, input_2, Weight_Q, Weight_K, Weight_V):
    """Host-side layout/precision prep (transposes, f32r rounding/splitting)."""
    i1t = np.ascontiguousarray(input_1.T)                 # [D, NQ]
    i1t_h, i1t_l = _split_f32r(i1t)
    wq_h, wq_l = _split_f32r(Weight_Q)
    wkt = np.ascontiguousarray(Weight_K.T)
    wkt_h, wkt_l = _split_f32r(wkt)
    i2t = np.ascontiguousarray(input_2.T)                 # [D, NK]
    i2t_h, i2t_l = _split_f32r(i2t)
    i2n = _round_f32r(input_2)
    wv = _round_f32r(Weight_V)

    in_maps = []
    for c in range(N_CORES):
        sl = slice(c * QS, (c + 1) * QS)
        in_maps.append({
            "i1t_h": np.ascontiguousarray(i1t_h[:, sl]),
            "i1t_l": np.ascontiguousarray(i1t_l[:, sl]),
            "wq_h": wq_h, "wq_l": wq_l,
            "wkt_h": wkt_h, "wkt_l": wkt_l,
            "i2t_h": i2t_h, "i2t_l": i2t_l,
            "i2n": i2n, "wv": wv,
        })
    return in_maps


def kernel(input_1, input_2, Weight_Q, Weight_K, Weight_V):
    global _compiled, _last_results
    from concourse import bass_utils

    if _compiled is None:
        _compiled = _build()
    nc = _compiled

    in_maps = _prep_inputs(input_1, input_2, Weight_Q, Weight_K, Weight_V)
    res = bass_utils.run_bass_kernel_spmd(nc, in_maps, core_ids=list(range(N_CORES)))
    _last_results = res
    return np.concatenate([res.results[c]["out"] for c in range(N_CORES)], axis=0)

